# revision 16
# baseline (speedup 1.0000x reference)
"""CapsuleLayer forward (squash + per-capsule matmul) on 8 Trainium2 cores.

Reference computation (all fp32):
    x  = inputs.reshape(B, 1152, 8)
    pc = squash(x)                              # per-(b,n) over k=8
    u_hat[b,n,j,d] = sum_k W[0,n,j,d,k] * pc[b,n,k]
    out = u_hat[..., None]                      # [B, 1152, 10, 16, 1]

Sharding: capsule dim (n=1152) split 144-per-core across 8 cores; every core
keeps the full batch (B=512).  Zero cross-device communication.

Per-core kernel (fp16 data paths; PSUM accumulates fp32).  Structure follows
the measured constraints of the machine:

  - K=128 (16-cap) flat block-diagonal W tiles [128, 2560] are mandatory
    for PE speed: K=64 matmuls never reach the 2.4GHz boost clock
    (measured 0/144 fast vs ~50% for K=128) and run a permanent 2x slower
    at 1.2GHz, which costs far more than the 16x-padded W's extra one-time
    HBM traffic (5.9MB/core)
  - only x chunk 0 dispatches on the scalar ring (a single DIRECT2D
    ahead of the ACT table load, so chunk-0's sqrt isn't flow-control
    blocked); W groups 0-8 and x chunks 1-3 all dispatch on the sync
    ring, whose queue holds nothing else until the first store.  All x
    chunks prefetch up-front
  - squash scale simplified algebraically: sq/((1+sq)*sqrt(sq+eps))
    == sqrt(sq)/(1+sq) (eps negligible, NaN-safe at sq=0)
  - NO GpSimd anywhere: every op a GpSimd chain feeds to DVE/ACT is a
    compile-time-schedulable stall bomb (the scheduler hoists the
    consumer above ready work and the queue head-of-line blocks for the
    slow GpSimd producer -- measured 11us of chunk-0 delay).  Instead
    ALL scale chains run on DVE (+ACT sqrt / 1+sq), which fits: DVE
    ~15.2us/chunk and ACT ~15.4 vs the 16.5us store window.  Chunks 1-3
    emit in two 72-cap bursts at the previous chunk's groups 3 and 6
  - pc transposed to [ck, b] via PE transpose (fp16 identity), pipelined
    one group ahead; PSUM->SBUF pcT copies alternate ACT/DVE
  - per group 5 512-col matmuls sharing one stationary pcT; PSUM->SBUF
    evacuation pa->DVE first (the next group's first matmul reuses that
    bank pair), pb->ACT, pcs alternating
  - output stored per group in [128, 2560] fp16 tiles ALTERNATING the
    sync/scalar HWDGE rings, so the SDMA engines round-robin two queue
    rows at packet granularity and the ~0.6us per-DMA completion-receipt
    bubble of a single-ring store stream is hidden
"""

from contextlib import ExitStack

import numpy as np

import concourse.bacc as bacc
import concourse.bass as bass  # noqa: F401  (AP helpers)
import concourse.mybir as mybir
import concourse.tile as tile
from concourse.bass_utils import run_bass_kernel_spmd
from concourse.masks import make_identity

N_CORES = 8
B = 512
N_CAPS = 1152
K = 8
JD = 160  # 10*16
CAPS_PER_CORE = N_CAPS // N_CORES  # 144
GROUP_CAPS = 16  # caps per matmul group -> K=128
N_GROUPS = CAPS_PER_CORE // GROUP_CAPS  # 9
GROUP_COLS = GROUP_CAPS * JD  # 2560
P = 128
B_CHUNKS = B // P  # 4

F32 = mybir.dt.float32
F16 = mybir.dt.float16
OUT_DT = mybir.dt.float16


def build_program():
    nc = bacc.Bacc("TRN2", debug=False, num_devices=N_CORES)
    x = nc.dram_tensor("x", [B, CAPS_PER_CORE * K], F16, kind="ExternalInput").ap()
    wb = nc.dram_tensor(
        "wb", [N_GROUPS * P, GROUP_COLS], F16, kind="ExternalInput"
    ).ap()
    out = nc.dram_tensor(
        "out", [B, CAPS_PER_CORE * JD], OUT_DT, kind="ExternalOutput"
    ).ap()

    with tile.TileContext(nc) as tc, ExitStack() as ctx:
        consts = ctx.enter_context(tc.tile_pool(name="consts", bufs=1))
        wblk_pool = ctx.enter_context(tc.tile_pool(name="wblk", bufs=1))
        xpool = ctx.enter_context(tc.tile_pool(name="xpool", bufs=4))
        x2pool = ctx.enter_context(tc.tile_pool(name="x2pool", bufs=2))
        pcpool = ctx.enter_context(tc.tile_pool(name="pcpool", bufs=2))
        stats = ctx.enter_context(tc.tile_pool(name="stats", bufs=2))
        pct_pool = ctx.enter_context(tc.tile_pool(name="pct", bufs=3))
        ost_pool = ctx.enter_context(tc.tile_pool(name="ost", bufs=6))
        # PSUM (8 banks): pa/pb rotate over three 2-bank slots (1.5
        # groups of slack), pcs gets its own 1-bank slot, transposes one.
        psum = ctx.enter_context(tc.tile_pool(name="psum", bufs=3, space="PSUM"))
        psum_c = ctx.enter_context(tc.tile_pool(name="psum_c", bufs=1, space="PSUM"))
        psum_t = ctx.enter_context(tc.tile_pool(name="psum_t", bufs=1, space="PSUM"))

        # ALL loads dispatch on the sync ring, x0 first -- the scalar
        # (ACT) queue stays clean for chunk-0's sqrt (its compiler-placed
        # table load would otherwise delay the x0 dispatch), and the sync
        # queue holds nothing else until the first store.
        xts = []
        xt0 = xpool.tile([P, CAPS_PER_CORE, K], F16, tag="xt0")
        nc.sync.dma_start(
            out=xt0, in_=x[0:P, :].rearrange("b (c k) -> b c k", k=K)
        )
        xts.append(xt0)
        wblk = []
        for g in range(N_GROUPS):
            wt = wblk_pool.tile(
                [P, GROUP_COLS], F16, tag=f"wblk{g}", name=f"wblk{g}"
            )
            nc.sync.dma_start(out=wt, in_=wb[g * P : (g + 1) * P, :])
            wblk.append(wt)
            if g < B_CHUNKS - 1:
                bi = g + 1
                xt = xpool.tile([P, CAPS_PER_CORE, K], F16, tag=f"xt{bi}")
                nc.sync.dma_start(
                    out=xt,
                    in_=x[bi * P : (bi + 1) * P, :].rearrange(
                        "b (c k) -> b c k", k=K
                    ),
                )
                xts.append(xt)

        identity = consts.tile([P, P], F16)
        make_identity(nc, identity)

        def emit_chain_dve(xt, pc, c0, c1):
            # DVE scale chain (+ACT sqrt and 1+sq) for caps [c0, c1).
            ncap = c1 - c0
            xs = xt[:, c0:c1, :]
            x2 = x2pool.tile([P, ncap, K], F16, tag=f"x2_{ncap}", name="x2")
            nc.scalar.activation(
                out=x2, in_=xs, func=mybir.ActivationFunctionType.Square
            )
            sq = stats.tile([P, ncap], F16, tag=f"sq_{ncap}", name="sq")
            nc.vector.reduce_sum(out=sq, in_=x2, axis=mybir.AxisListType.X)
            sn = stats.tile([P, ncap], F16, tag=f"sn_{ncap}", name="sn")
            nc.scalar.activation(
                out=sn, in_=sq, func=mybir.ActivationFunctionType.Sqrt
            )
            t1 = stats.tile([P, ncap], F16, tag=f"t1_{ncap}", name="t1")
            nc.scalar.activation(
                out=t1, in_=sq, func=mybir.ActivationFunctionType.Identity,
                bias=1.0,
            )
            rden = stats.tile([P, ncap], F16, tag=f"rd_{ncap}", name="rd")
            nc.vector.reciprocal(rden, t1)
            scale = stats.tile([P, ncap], F16, tag=f"sc_{ncap}", name="sc")
            nc.vector.tensor_mul(scale, sn, rden)
            nc.vector.tensor_mul(
                pc[:, c0:c1, :],
                xs,
                scale.unsqueeze(2).broadcast_to([P, ncap, K]),
            )

        def issue_transpose(pc_flat, g):
            # Pipelined one group ahead so the PE never waits on the
            # PSUM->SBUF pcT copy.
            pst = psum_t.tile([P, P], F16, tag="pt")
            nc.tensor.transpose(pst, pc_flat[:, g * P : (g + 1) * P], identity)
            pcT = pct_pool.tile([P, P], F16, tag="pcT", name="pcT")
            if g % 2 == 0:
                nc.scalar.copy(pcT, pst)
            else:
                nc.vector.tensor_copy(pcT, pst)
            return pcT

        with nc.allow_low_precision("fp16 squash: tolerance is 2e-2"):
            pc0 = pcpool.tile([P, CAPS_PER_CORE, K], F16, tag="pc")
            emit_chain_dve(xts[0], pc0, 0, 2 * GROUP_CAPS)
            emit_chain_dve(xts[0], pc0, 2 * GROUP_CAPS, CAPS_PER_CORE)

            pc_cur = pc0.rearrange("p c k -> p (c k)")
            pc_next = None
            pcn = None
            # Transposes are issued TWO groups ahead (their PSUM->SBUF
            # pcT copies queue behind ~1.1us evac ops on DVE/ACT; one
            # group of lead time was measured to stall the PE ~1.3us on
            # every other group).  tq holds the pending pcT tiles.
            tq = [issue_transpose(pc_cur, 0), issue_transpose(pc_cur, 1)]
            for bi in range(B_CHUNKS):
                for g in range(N_GROUPS):
                    pcT = tq.pop(0)

                    if bi + 1 < B_CHUNKS:
                        # Next chunk's chain in two DVE bursts; inputs are
                        # already resident, so no queue ever stalls.  g==5
                        # finishes well before this chunk's group-7 issue
                        # of the next chunk's first transpose.
                        if g == 2:
                            pcn = pcpool.tile(
                                [P, CAPS_PER_CORE, K], F16, tag="pc"
                            )
                            emit_chain_dve(
                                xts[bi + 1], pcn, 0, CAPS_PER_CORE // 2
                            )
                            pc_next = pcn.rearrange("p c k -> p (c k)")
                        elif g == 5:
                            emit_chain_dve(
                                xts[bi + 1], pcn, CAPS_PER_CORE // 2,
                                CAPS_PER_CORE,
                            )

                    # The small pcs piece runs FIRST: the next group's
                    # first matmul then depends on the smallest,
                    # earliest-evacuated PSUM slot instead of the 2-bank
                    # pa (measured ~1.2us stall on every other group).
                    pcs = psum_c.tile([P, 512], F32, tag="pcs", name="pcs")
                    pa = psum.tile([P, 1024], F32, tag="pm")
                    pb = psum.tile([P, 1024], F32, tag="pm")
                    nc.tensor.matmul(
                        pcs, lhsT=pcT, rhs=wblk[g][:, 4 * 512 : 5 * 512],
                        start=True, stop=True,
                    )
                    for s in range(2):
                        nc.tensor.matmul(
                            pa[:, s * 512 : (s + 1) * 512],
                            lhsT=pcT,
                            rhs=wblk[g][:, s * 512 : (s + 1) * 512],
                            start=True,
                            stop=True,
                        )
                    for s in range(2):
                        nc.tensor.matmul(
                            pb[:, s * 512 : (s + 1) * 512],
                            lhsT=pcT,
                            rhs=wblk[g][:, (2 + s) * 512 : (3 + s) * 512],
                            start=True,
                            stop=True,
                        )

                    ost = ost_pool.tile([P, GROUP_COLS], OUT_DT)
                    if g % 2 == 0:
                        nc.vector.tensor_copy(ost[:, 2048:2560], pcs)
                    else:
                        nc.scalar.copy(ost[:, 2048:2560], pcs)
                    nc.vector.tensor_copy(ost[:, 0:1024], pa)
                    nc.scalar.copy(ost[:, 1024:2048], pb)
                    # Alternate sync/gpsimd rings: two queue rows for the
                    # SDMA round-robin, and neither dispatch queue carries
                    # compute (scalar-ring dispatches cost the ACT queue
                    # ~0.7us each).
                    st_eng = nc.sync if (bi * N_GROUPS + g) % 2 == 0 else nc.gpsimd
                    st_eng.dma_start(
                        out=out[
                            bi * P : (bi + 1) * P,
                            g * GROUP_COLS : (g + 1) * GROUP_COLS,
                        ],
                        in_=ost,
                    )

                    # Issue the transpose two groups ahead.
                    ng, nbi = g + 2, bi
                    if ng >= N_GROUPS:
                        ng, nbi = ng - N_GROUPS, bi + 1
                    if nbi < B_CHUNKS:
                        src_pc = pc_cur if nbi == bi else pc_next
                        tq.append(issue_transpose(src_pc, ng))
                pc_cur = pc_next
    nc.compile()
    return nc


_PROGRAM = None


def _get_program():
    global _PROGRAM
    if _PROGRAM is None:
        _PROGRAM = build_program()
    return _PROGRAM


def shard_inputs(inputs: np.ndarray, W: np.ndarray) -> list[dict[str, np.ndarray]]:
    # Flat K=128 16-cap block-diagonal W per core: per group one
    # [128, 2560] tile; rows (c,k) = c*8+k with W[n=c] at cols
    # c*160..(c+1)*160, zeros elsewhere.
    w0 = np.asarray(W[0], dtype=np.float32).reshape(N_CAPS, JD, K)
    x16 = np.asarray(inputs, dtype=np.float16)
    in_maps = []
    for i in range(N_CORES):
        c0 = i * CAPS_PER_CORE
        wcore = w0[c0 : c0 + CAPS_PER_CORE]  # [144, 160, 8]
        wbd = np.zeros(
            (N_GROUPS, GROUP_CAPS, K, GROUP_CAPS, JD), dtype=np.float16
        )
        for c in range(GROUP_CAPS):
            # wbd[g, c, k, c, jd] = W[g*16+c, jd, k]
            wbd[:, c, :, c, :] = (
                wcore.reshape(N_GROUPS, GROUP_CAPS, JD, K)[:, c]
                .transpose(0, 2, 1)
            )
        in_maps.append(
            {
                "x": np.ascontiguousarray(
                    x16[:, c0 * K : (c0 + CAPS_PER_CORE) * K]
                ),
                "wb": wbd.reshape(N_GROUPS * P, GROUP_COLS),
            }
        )
    return in_maps


def unshard_output(results: list[dict[str, np.ndarray]]) -> np.ndarray:
    full = np.empty((B, N_CAPS, JD), dtype=np.float32)
    for i in range(N_CORES):
        c0 = i * CAPS_PER_CORE
        full[:, c0 : c0 + CAPS_PER_CORE, :] = results[i]["out"].reshape(
            B, CAPS_PER_CORE, JD
        ).astype(np.float32)
    return full.reshape(B, N_CAPS, 10, 16, 1)


def kernel(inputs: np.ndarray, W: np.ndarray) -> np.ndarray:
    nc = _get_program()
    in_maps = shard_inputs(np.asarray(inputs), np.asarray(W))
    res = run_bass_kernel_spmd(nc, in_maps, core_ids=list(range(N_CORES)))
    return unshard_output(res.results)


# revision 17
# speedup vs baseline: 1.1019x; 1.1019x over previous
"""CapsuleLayer forward (squash + per-capsule matmul) on 8 Trainium2 cores.

Reference computation (all fp32):
    x  = inputs.reshape(B, 1152, 8)
    pc = squash(x)                              # per-(b,n) over k=8
    u_hat[b,n,j,d] = sum_k W[0,n,j,d,k] * pc[b,n,k]
    out = u_hat[..., None]                      # [B, 1152, 10, 16, 1]

Sharding: capsule dim (n=1152) split 144-per-core across 8 cores; every core
keeps the full batch (B=512).  Zero cross-device communication.

Per-core kernel (fp16 data paths; PSUM accumulates fp32).  Structure follows
the measured constraints of the machine:

  - K=128 (16-cap) flat block-diagonal W tiles [128, 2560] are mandatory
    for PE speed: K=64 matmuls never reach the 2.4GHz boost clock
    (measured 0/144 fast vs ~50% for K=128) and run a permanent 2x slower
    at 1.2GHz, which costs far more than the 16x-padded W's extra one-time
    HBM traffic (5.9MB/core)
  - only x chunk 0 dispatches on the scalar ring (a single DIRECT2D
    ahead of the ACT table load, so chunk-0's sqrt isn't flow-control
    blocked); W groups 0-8 and x chunks 1-3 all dispatch on the sync
    ring, whose queue holds nothing else until the first store.  All x
    chunks prefetch up-front
  - squash scale simplified algebraically: sq/((1+sq)*sqrt(sq+eps))
    == sqrt(sq)/(1+sq) (eps negligible, NaN-safe at sq=0)
  - NO GpSimd anywhere: every op a GpSimd chain feeds to DVE/ACT is a
    compile-time-schedulable stall bomb (the scheduler hoists the
    consumer above ready work and the queue head-of-line blocks for the
    slow GpSimd producer -- measured 11us of chunk-0 delay).  Instead
    ALL scale chains run on DVE (+ACT sqrt / 1+sq), which fits: DVE
    ~15.2us/chunk and ACT ~15.4 vs the 16.5us store window.  Chunks 1-3
    emit in two 72-cap bursts at the previous chunk's groups 3 and 6
  - pc transposed to [ck, b] via PE transpose (fp16 identity), pipelined
    one group ahead; PSUM->SBUF pcT copies alternate ACT/DVE
  - per group 5 512-col matmuls sharing one stationary pcT; PSUM->SBUF
    evacuation pa->DVE first (the next group's first matmul reuses that
    bank pair), pb->ACT, pcs alternating
  - output stored per group in [128, 2560] fp16 tiles ALTERNATING the
    sync/scalar HWDGE rings, so the SDMA engines round-robin two queue
    rows at packet granularity and the ~0.6us per-DMA completion-receipt
    bubble of a single-ring store stream is hidden
"""

from contextlib import ExitStack

import numpy as np

import concourse.bacc as bacc
import concourse.bass as bass  # noqa: F401  (AP helpers)
import concourse.mybir as mybir
import concourse.tile as tile
from concourse.bass_utils import run_bass_kernel_spmd
from concourse.masks import make_identity

N_CORES = 8
B = 512
N_CAPS = 1152
K = 8
JD = 160  # 10*16
CAPS_PER_CORE = N_CAPS // N_CORES  # 144
GROUP_CAPS = 16  # caps per matmul group -> K=128
N_GROUPS = CAPS_PER_CORE // GROUP_CAPS  # 9
GROUP_COLS = GROUP_CAPS * JD  # 2560
P = 128
B_CHUNKS = B // P  # 4

F32 = mybir.dt.float32
F16 = mybir.dt.float16
OUT_DT = mybir.dt.float16


def build_program():
    nc = bacc.Bacc("TRN2", debug=False, num_devices=N_CORES)
    x = nc.dram_tensor("x", [B, CAPS_PER_CORE * K], F16, kind="ExternalInput").ap()
    wb = nc.dram_tensor(
        "wb", [N_GROUPS * P, GROUP_COLS], F16, kind="ExternalInput"
    ).ap()
    out = nc.dram_tensor(
        "out", [B, CAPS_PER_CORE * JD], OUT_DT, kind="ExternalOutput"
    ).ap()

    with tile.TileContext(nc) as tc, ExitStack() as ctx:
        consts = ctx.enter_context(tc.tile_pool(name="consts", bufs=1))
        wblk_pool = ctx.enter_context(tc.tile_pool(name="wblk", bufs=1))
        xpool = ctx.enter_context(tc.tile_pool(name="xpool", bufs=4))
        x2pool = ctx.enter_context(tc.tile_pool(name="x2pool", bufs=2))
        pcpool = ctx.enter_context(tc.tile_pool(name="pcpool", bufs=2))
        stats = ctx.enter_context(tc.tile_pool(name="stats", bufs=2))
        pct_pool = ctx.enter_context(tc.tile_pool(name="pct", bufs=3))
        ost_pool = ctx.enter_context(tc.tile_pool(name="ost", bufs=6))
        # PSUM: 3x 2-bank matmul slots + 2x 1-bank transpose slots = 8 banks.
        psum = ctx.enter_context(tc.tile_pool(name="psum", bufs=3, space="PSUM"))
        psum_t = ctx.enter_context(tc.tile_pool(name="psum_t", bufs=2, space="PSUM"))

        # ALL loads dispatch on the sync ring, x0 first -- the scalar
        # (ACT) queue stays clean for chunk-0's sqrt (its compiler-placed
        # table load would otherwise delay the x0 dispatch), and the sync
        # queue holds nothing else until the first store.
        xts = []
        xt0 = xpool.tile([P, CAPS_PER_CORE, K], F16, tag="xt0")
        nc.sync.dma_start(
            out=xt0, in_=x[0:P, :].rearrange("b (c k) -> b c k", k=K)
        )
        xts.append(xt0)
        wblk = []
        for g in range(N_GROUPS):
            wt = wblk_pool.tile(
                [P, GROUP_COLS], F16, tag=f"wblk{g}", name=f"wblk{g}"
            )
            nc.sync.dma_start(out=wt, in_=wb[g * P : (g + 1) * P, :])
            wblk.append(wt)
            if g < B_CHUNKS - 1:
                bi = g + 1
                xt = xpool.tile([P, CAPS_PER_CORE, K], F16, tag=f"xt{bi}")
                nc.sync.dma_start(
                    out=xt,
                    in_=x[bi * P : (bi + 1) * P, :].rearrange(
                        "b (c k) -> b c k", k=K
                    ),
                )
                xts.append(xt)

        identity = consts.tile([P, P], F16)
        make_identity(nc, identity)

        def emit_chain_dve(xt, pc, c0, c1):
            # DVE scale chain (+ACT sqrt and 1+sq) for caps [c0, c1).
            ncap = c1 - c0
            xs = xt[:, c0:c1, :]
            x2 = x2pool.tile([P, ncap, K], F16, tag=f"x2_{ncap}", name="x2")
            nc.vector.tensor_mul(x2, xs, xs)
            sq = stats.tile([P, ncap], F16, tag=f"sq_{ncap}", name="sq")
            nc.vector.reduce_sum(out=sq, in_=x2, axis=mybir.AxisListType.X)
            sn = stats.tile([P, ncap], F16, tag=f"sn_{ncap}", name="sn")
            nc.scalar.activation(
                out=sn, in_=sq, func=mybir.ActivationFunctionType.Sqrt
            )
            t1 = stats.tile([P, ncap], F16, tag=f"t1_{ncap}", name="t1")
            nc.scalar.activation(
                out=t1, in_=sq, func=mybir.ActivationFunctionType.Identity,
                bias=1.0,
            )
            rden = stats.tile([P, ncap], F16, tag=f"rd_{ncap}", name="rd")
            nc.vector.reciprocal(rden, t1)
            scale = stats.tile([P, ncap], F16, tag=f"sc_{ncap}", name="sc")
            nc.vector.tensor_mul(scale, sn, rden)
            nc.vector.tensor_mul(
                pc[:, c0:c1, :],
                xs,
                scale.unsqueeze(2).broadcast_to([P, ncap, K]),
            )

        def issue_transpose(pc_flat, g):
            # Pipelined one group ahead so the PE never waits on the
            # PSUM->SBUF pcT copy.
            pst = psum_t.tile([P, P], F16, tag="pt")
            nc.tensor.transpose(pst, pc_flat[:, g * P : (g + 1) * P], identity)
            pcT = pct_pool.tile([P, P], F16, tag="pcT", name="pcT")
            if g % 2 == 0:
                nc.scalar.copy(pcT, pst)
            else:
                nc.vector.tensor_copy(pcT, pst)
            return pcT

        with nc.allow_low_precision("fp16 squash: tolerance is 2e-2"):
            pc0 = pcpool.tile([P, CAPS_PER_CORE, K], F16, tag="pc")
            emit_chain_dve(xts[0], pc0, 0, 2 * GROUP_CAPS)
            emit_chain_dve(xts[0], pc0, 2 * GROUP_CAPS, CAPS_PER_CORE)

            pc_cur = pc0.rearrange("p c k -> p (c k)")
            pc_next = None
            pcn = None
            # Transposes are issued TWO groups ahead (their PSUM->SBUF
            # pcT copies queue behind ~1.1us evac ops on DVE/ACT; one
            # group of lead time was measured to stall the PE ~1.3us on
            # every other group).  tq holds the pending pcT tiles.
            tq = [issue_transpose(pc_cur, 0), issue_transpose(pc_cur, 1)]
            for bi in range(B_CHUNKS):
                for g in range(N_GROUPS):
                    pcT = tq.pop(0)

                    if bi + 1 < B_CHUNKS:
                        # Next chunk's chain in two DVE bursts; inputs are
                        # already resident, so no queue ever stalls.  g==5
                        # finishes well before this chunk's group-7 issue
                        # of the next chunk's first transpose.
                        if g == 2:
                            pcn = pcpool.tile(
                                [P, CAPS_PER_CORE, K], F16, tag="pc"
                            )
                            emit_chain_dve(
                                xts[bi + 1], pcn, 0, CAPS_PER_CORE // 2
                            )
                            pc_next = pcn.rearrange("p c k -> p (c k)")
                        elif g == 5:
                            emit_chain_dve(
                                xts[bi + 1], pcn, CAPS_PER_CORE // 2,
                                CAPS_PER_CORE,
                            )

                    # The small pcs piece runs FIRST: the next group's
                    # first matmul then depends on the smallest,
                    # earliest-evacuated PSUM slot instead of the 2-bank
                    # pa (measured ~1.2us stall on every other group).
                    pcs = psum.tile([P, 512], F32, tag="pm")
                    pa = psum.tile([P, 1024], F32, tag="pm")
                    pb = psum.tile([P, 1024], F32, tag="pm")
                    nc.tensor.matmul(
                        pcs, lhsT=pcT, rhs=wblk[g][:, 4 * 512 : 5 * 512],
                        start=True, stop=True,
                    )
                    for s in range(2):
                        nc.tensor.matmul(
                            pa[:, s * 512 : (s + 1) * 512],
                            lhsT=pcT,
                            rhs=wblk[g][:, s * 512 : (s + 1) * 512],
                            start=True,
                            stop=True,
                        )
                    for s in range(2):
                        nc.tensor.matmul(
                            pb[:, s * 512 : (s + 1) * 512],
                            lhsT=pcT,
                            rhs=wblk[g][:, (2 + s) * 512 : (3 + s) * 512],
                            start=True,
                            stop=True,
                        )

                    ost = ost_pool.tile([P, GROUP_COLS], OUT_DT)
                    if g % 2 == 0:
                        nc.vector.tensor_copy(ost[:, 2048:2560], pcs)
                    else:
                        nc.scalar.copy(ost[:, 2048:2560], pcs)
                    nc.vector.tensor_copy(ost[:, 0:1024], pa)
                    nc.scalar.copy(ost[:, 1024:2048], pb)
                    # Alternate sync/gpsimd rings: two queue rows for the
                    # SDMA round-robin, and neither dispatch queue carries
                    # compute (scalar-ring dispatches cost the ACT queue
                    # ~0.7us each).
                    st_eng = nc.sync if (bi * N_GROUPS + g) % 2 == 0 else nc.gpsimd
                    st_eng.dma_start(
                        out=out[
                            bi * P : (bi + 1) * P,
                            g * GROUP_COLS : (g + 1) * GROUP_COLS,
                        ],
                        in_=ost,
                    )

                    # Issue the transpose two groups ahead.
                    ng, nbi = g + 2, bi
                    if ng >= N_GROUPS:
                        ng, nbi = ng - N_GROUPS, bi + 1
                    if nbi < B_CHUNKS:
                        src_pc = pc_cur if nbi == bi else pc_next
                        tq.append(issue_transpose(src_pc, ng))
                pc_cur = pc_next
    nc.compile()
    return nc


_PROGRAM = None


def _get_program():
    global _PROGRAM
    if _PROGRAM is None:
        _PROGRAM = build_program()
    return _PROGRAM


def shard_inputs(inputs: np.ndarray, W: np.ndarray) -> list[dict[str, np.ndarray]]:
    # Flat K=128 16-cap block-diagonal W per core: per group one
    # [128, 2560] tile; rows (c,k) = c*8+k with W[n=c] at cols
    # c*160..(c+1)*160, zeros elsewhere.
    w0 = np.asarray(W[0], dtype=np.float32).reshape(N_CAPS, JD, K)
    x16 = np.asarray(inputs, dtype=np.float16)
    in_maps = []
    for i in range(N_CORES):
        c0 = i * CAPS_PER_CORE
        wcore = w0[c0 : c0 + CAPS_PER_CORE]  # [144, 160, 8]
        wbd = np.zeros(
            (N_GROUPS, GROUP_CAPS, K, GROUP_CAPS, JD), dtype=np.float16
        )
        for c in range(GROUP_CAPS):
            # wbd[g, c, k, c, jd] = W[g*16+c, jd, k]
            wbd[:, c, :, c, :] = (
                wcore.reshape(N_GROUPS, GROUP_CAPS, JD, K)[:, c]
                .transpose(0, 2, 1)
            )
        in_maps.append(
            {
                "x": np.ascontiguousarray(
                    x16[:, c0 * K : (c0 + CAPS_PER_CORE) * K]
                ),
                "wb": wbd.reshape(N_GROUPS * P, GROUP_COLS),
            }
        )
    return in_maps


def unshard_output(results: list[dict[str, np.ndarray]]) -> np.ndarray:
    full = np.empty((B, N_CAPS, JD), dtype=np.float32)
    for i in range(N_CORES):
        c0 = i * CAPS_PER_CORE
        full[:, c0 : c0 + CAPS_PER_CORE, :] = results[i]["out"].reshape(
            B, CAPS_PER_CORE, JD
        ).astype(np.float32)
    return full.reshape(B, N_CAPS, 10, 16, 1)


def kernel(inputs: np.ndarray, W: np.ndarray) -> np.ndarray:
    nc = _get_program()
    in_maps = shard_inputs(np.asarray(inputs), np.asarray(W))
    res = run_bass_kernel_spmd(nc, in_maps, core_ids=list(range(N_CORES)))
    return unshard_output(res.results)
